# revision 16
# baseline (speedup 1.0000x reference)
"""2-layer GRU (T=512, B=64, E=300, H=512) on 8 NeuronCores.

Strategy v2: 4-way batch-parallel x 2-way layer-pipeline.
  - cores 0-3: layer 0 on batch quarter q=core (BC=16 seqs each)
  - cores 4-7: layer 1 on batch quarter q=core-4, lagging one 32-step
    chunk; layer-0 output chunks hop over via paired AllGathers.
  - One uniform SPMD program; the layer split is pure data: per-core
    weights, with zero-weight + phase-gate tricks keeping the inactive
    paths exactly zero (so layer-1 state stays 0 through its idle
    phase 0 and program control flow never diverges).
Per step (t = 544 phases*32): 52 matmul instructions, gate-ordered
  r(16) -> n(16+4 bias-MMs) -> z(16), with an 8-op chain
  (ar,sig_r | hm,an,tanh | az,sig_z | d,e,h') scheduled so the chain
  head overlaps the n/z matmul groups.
Host: final 4-way partial-sum over layer-1 cores, mean, tiny FC.
"""
import numpy as np
import ml_dtypes

T, B, E, H, V, L = 512, 64, 300, 512, 30000, 5
NCORE = 8
DP = 4                    # batch-parallel width
BC = B // DP              # 16 sequences per core
CH = 32                   # steps per chunk/phase
NCH = T // CH             # 16 data chunks
NPH = NCH + 2             # 18 phases (layer-1 lags two chunks)
G3 = 3 * H                # 1536
KH = H // 128             # 4 k-tiles (contraction)
M3 = G3 // 128            # 12 m-tiles (output gates)
CB = CH * BC              # 512 cols per chunk
SEG = (NPH * CH + 1) * BC  # 8720 cols per k-segment of state
NBLK = T * BC // 128      # 64 gather blocks of 128 tokens
EK = 3                    # E=300 -> 3 partial k-tiles (128,128,44)

_cache = {}


def _build():
    from contextlib import ExitStack
    import concourse.bass as bass
    import concourse.mybir as mybir
    import concourse.tile as tile
    from concourse import bacc
    from concourse.masks import make_identity

    bf16, f32, i32 = mybir.dt.bfloat16, mybir.dt.float32, mybir.dt.int32
    AF = mybir.ActivationFunctionType
    OP = mybir.AluOpType

    nc = bacc.Bacc("TRN2", target_bir_lowering=False, debug=False,
                   num_devices=NCORE)
    emb_d = nc.dram_tensor("emb", [V, E], f32, kind="ExternalInput").ap()
    idx_d = nc.dram_tensor("idx", [128, NBLK], i32, kind="ExternalInput").ap()
    whh_d = nc.dram_tensor("whh", [128, KH * G3], bf16, kind="ExternalInput").ap()
    wia_d = nc.dram_tensor("wia", [128, EK * G3], bf16, kind="ExternalInput").ap()
    wib_d = nc.dram_tensor("wib", [128, KH * G3], bf16, kind="ExternalInput").ap()
    b0_d = nc.dram_tensor("b0", [128, NPH * M3], f32, kind="ExternalInput").ap()
    bn_d = nc.dram_tensor("bn", [128, KH * 128], bf16, kind="ExternalInput").ap()
    pg_d = nc.dram_tensor("pg", [128, NPH * BC], bf16, kind="ExternalInput").ap()
    out_d = nc.dram_tensor("out", [128, KH * T], f32, kind="ExternalOutput").ap()
    # collective bounce buffers (double-buffered across phases)
    snd_d = [nc.dram_tensor(f"snd{i}", [128, KH * CB], bf16) for i in range(2)]
    rcv_d = [nc.dram_tensor(f"rcv{i}", [2, 128, KH * CB], bf16) for i in range(2)]
    GROUPS = [[c, c + DP] for c in range(DP)]

    with tile.TileContext(nc) as tc, ExitStack() as ctx:
        wp = ctx.enter_context(tc.tile_pool(name="wp", bufs=1))
        sp = ctx.enter_context(tc.tile_pool(name="sp", bufs=1))
        xb = ctx.enter_context(tc.tile_pool(name="xb", bufs=2))
        gb = ctx.enter_context(tc.tile_pool(name="gb", bufs=2))
        rxp = ctx.enter_context(tc.tile_pool(name="rxp", bufs=2))
        tp = ctx.enter_context(tc.tile_pool(name="tp", bufs=3))
        pp = ctx.enter_context(tc.tile_pool(name="pp", bufs=1, space="PSUM"))
        px = ctx.enter_context(tc.tile_pool(name="px", bufs=2, space="PSUM"))
        pt = ctx.enter_context(tc.tile_pool(name="pt", bufs=2, space="PSUM"))

        def wtile(nm, shape, dt, src=None):
            t = wp.tile(shape, dt, name=nm, tag=nm)
            if src is not None:
                nc.sync.dma_start(out=t[:], in_=src[:])
            return t

        whh = wtile("whh_t", [128, KH * G3], bf16, whh_d)
        wia = wtile("wia_t", [128, EK * G3], bf16, wia_d)
        wib = wtile("wib_t", [128, KH * G3], bf16, wib_d)
        b0 = wtile("b0_t", [128, NPH * M3], f32, b0_d)
        bn = wtile("bn_t", [128, KH * 128], bf16, bn_d)
        pg = wtile("pg_t", [128, NPH * BC], bf16, pg_d)
        idx_t = wtile("idx_t", [128, NBLK], i32, idx_d)
        ident = wtile("ident", [128, 128], bf16)
        make_identity(nc, ident[:])

        st = sp.tile([128, KH * SEG], bf16, name="st", tag="st")
        pooled = sp.tile([128, KH * T], f32, name="pooled", tag="pooled")
        for k in range(KH):
            nc.vector.memset(st[:, k * SEG:k * SEG + BC], 0.0)
        rxT = []
        for i in range(2):
            r = sp.tile([128, KH * CB], bf16, name=f"rxT{i}", tag=f"rxT{i}")
            nc.vector.memset(r[:], 0.0)
            rxT.append(r)

        def gather(c, xT):
            """indirect-gather + transpose chunk c tokens into xT buffer"""
            for j in range(4):
                blk = 4 * c + j
                xg = tp.tile([128, E], f32, name="xg", tag="xg")
                nc.gpsimd.indirect_dma_start(
                    out=xg[:], out_offset=None, in_=emb_d[:],
                    in_offset=bass.IndirectOffsetOnAxis(
                        ap=idx_t[:, blk:blk + 1], axis=0))
                xc = tp.tile([128, E], bf16, name="xc", tag="xc")
                nc.vector.tensor_copy(out=xc[:], in_=xg[:])
                for e in range(EK):
                    ke = min(128, E - e * 128)
                    tps = pt.tile([128, 128], bf16, name="tps", tag="tps")
                    nc.tensor.transpose(out=tps[0:ke, :],
                                        in_=xc[:, e * 128:e * 128 + ke],
                                        identity=ident[:])
                    nc.vector.tensor_copy(
                        out=xT[0:ke, e * CB + j * 128:e * CB + (j + 1) * 128],
                        in_=tps[0:ke, :])

        XPORD = [0, 1, 2, 3, 8, 9, 10, 11, 4, 5, 6, 7]

        def xp_group(p, i, xT, rx, xpb):
            """one input-projection m-group for phase p (7 MMs + bias copy).
            m-order r,n,z matches first-step consumption order; the
            PSUM->SBUF bias-copies alternate ACT/VE so neither queue
            blocks the steps' chain ops."""
            m = XPORD[i]
            xpp = px.tile([128, CB], f32, name="xpp", tag="xpp")
            first = True
            for e in range(EK):
                ke = min(128, E - e * 128)
                nc.tensor.matmul(
                    out=xpp[:, 0:CB],
                    lhsT=wia[0:ke, e * G3 + m * 128:e * G3 + (m + 1) * 128],
                    rhs=xT[0:ke, e * CB:(e + 1) * CB],
                    start=first, stop=False)
                first = False
            for k in range(KH):
                nc.tensor.matmul(
                    out=xpp[:, 0:CB],
                    lhsT=wib[:, k * G3 + m * 128:k * G3 + (m + 1) * 128],
                    rhs=rx[:, k * CB:(k + 1) * CB],
                    start=False, stop=(k == KH - 1))
            if i % 2 == 0:
                nc.scalar.activation(out=xpb[:, m * CB:(m + 1) * CB],
                                     in_=xpp[:, 0:CB], func=AF.Identity,
                                     bias=b0[:, p * M3 + m:p * M3 + m + 1])
            else:
                nc.vector.tensor_scalar_add(
                    out=xpb[:, m * CB:(m + 1) * CB], in0=xpp[:, 0:CB],
                    scalar1=b0[:, p * M3 + m:p * M3 + m + 1])

        def xp_alloc():
            return rxp.tile([128, M3 * CB], bf16, name="xpb", tag="xpb")

        def step(p, ts, xpb):
            t = p * CH + ts
            # full-bank tiles: PSUM visibility is bank-granular, so each
            # gate gets its own 2KB bank to keep readers ungated by the
            # other gates' matmuls
            gr = pp.tile([128, 512], f32, name="gr", tag="gr")
            gz = pp.tile([128, 512], f32, name="gz", tag="gz")
            gn = pp.tile([128, 512], f32, name="gn", tag="gn")

            def mm_group(g, gate, bias_mm):
                for mi in range(4):
                    m = gate * 4 + mi
                    for k in range(KH):
                        nc.tensor.matmul(
                            out=g[:, mi * BC:(mi + 1) * BC],
                            lhsT=whh[:, k * G3 + m * 128:k * G3 + (m + 1) * 128],
                            rhs=st[:, k * SEG + t * BC:k * SEG + (t + 1) * BC],
                            start=(k == 0),
                            stop=(k == KH - 1 and not bias_mm))
                    if bias_mm:
                        nc.tensor.matmul(
                            out=g[:, mi * BC:(mi + 1) * BC],
                            lhsT=bn[0:1, mi * 128:(mi + 1) * 128],
                            rhs=pg[0:1, p * BC:(p + 1) * BC],
                            start=False, stop=True)

            xpv = xpb[:].rearrange("p (m s) -> p m s", m=M3)
            sl = ts * BC
            stv = st[:].rearrange("p (c s) -> p c s", c=KH)

            def tmp(nm, dt=f32):
                return tp.tile([128, KH * BC], dt, name=nm, tag=nm)

            # r group
            mm_group(gr, 0, False)
            ar = tmp("ar")
            nc.vector.tensor_tensor(
                out=ar[:].rearrange("p (m b) -> p m b", b=BC),
                in0=xpv[:, 0:4, sl:sl + BC],
                in1=gr[:, 0:4 * BC].rearrange("p (m b) -> p m b", b=BC), op=OP.add)
            r = tmp("r", bf16)
            nc.scalar.activation(out=r[:], in_=ar[:], func=AF.Sigmoid)
            # n group (hidden-side bias via phase-gated rank-4 matmul)
            mm_group(gn, 2, True)
            hm = tmp("hm")
            nc.vector.tensor_tensor(out=hm[:], in0=r[:],
                                    in1=gn[:, 0:4 * BC], op=OP.mult)
            an = tmp("an")
            nc.vector.tensor_tensor(
                out=an[:].rearrange("p (m b) -> p m b", b=BC),
                in0=xpv[:, 8:12, sl:sl + BC],
                in1=hm[:].rearrange("p (m b) -> p m b", b=BC), op=OP.add)
            n = tmp("n", bf16)
            nc.scalar.activation(out=n[:], in_=an[:], func=AF.Tanh)
            # d emitted before the z-gate ops so the in-order Vector queue
            # runs [ar, hm, an, d, az, e, h'] -- az (gated by the z matmuls,
            # which finish last) must not block the n path
            d = tmp("d")
            nc.vector.tensor_tensor(
                out=d[:].rearrange("p (c b) -> p c b", c=KH),
                in0=stv[:, :, t * BC:(t + 1) * BC],
                in1=n[:].rearrange("p (c b) -> p c b", c=KH), op=OP.subtract)
            # z group
            mm_group(gz, 1, False)
            az = tmp("az")
            # az is gated by the z matmuls (last PE group); pin it late in
            # the in-order Vector queue via the scheduler's virtual clock,
            # else the scheduler (whose cost model thinks matmuls are
            # near-instant) hoists it ahead of hm/an/d and blocks the n path
            with tc.tile_wait_until((4 * t + 3) / 1000.0):
                nc.vector.tensor_tensor(
                    out=az[:].rearrange("p (m b) -> p m b", b=BC),
                    in0=xpv[:, 4:8, sl:sl + BC],
                    in1=gz[:, 0:4 * BC].rearrange("p (m b) -> p m b", b=BC), op=OP.add)
            z = tmp("z", bf16)
            nc.scalar.activation(out=z[:], in_=az[:], func=AF.Sigmoid)
            # h' = n + z * (h - n)
            e_ = tmp("e")
            nc.vector.tensor_tensor(out=e_[:], in0=z[:], in1=d[:], op=OP.mult)
            nc.vector.tensor_tensor(
                out=stv[:, :, (t + 1) * BC:(t + 2) * BC],
                in0=n[:].rearrange("p (c b) -> p c b", c=KH),
                in1=e_[:].rearrange("p (c b) -> p c b", c=KH), op=OP.add)

        # ---- main schedule ----
        # xp for phase p+1 is interleaved into phase p's step gaps (the PE
        # would otherwise idle there waiting on the gate chain, and the
        # steady matmul stream keeps the HAM clock-gate warm); gathers for
        # chunk p+2 are emitted mid-phase.
        xTb = []
        for i in range(2):
            xTb.append(xb.tile([128, EK * CB], bf16, name=f"xT{i}", tag=f"xT{i}"))
        gather(0, xTb[0])
        gather(1, xTb[1])
        xpb_cur = xp_alloc()
        for i in range(M3):
            xp_group(0, i, xTb[0], rxT[0], xpb_cur)
        for p in range(NPH):
            xpb_next = xp_alloc() if p + 1 < NPH else None
            for ts in range(CH):
                step(p, ts, xpb_cur)
                if xpb_next is not None and ts >= 8 and ts % 2 == 0:
                    xp_group(p + 1, (ts - 8) // 2, xTb[(p + 1) % 2],
                             rxT[(p + 1) % 2], xpb_next)
                if ts == 17 and p + 2 < NCH:
                    gather(p + 2, xTb[(p + 2) % 2])
            if p < NCH:
                io = p % 2
                stv = st[:].rearrange("p (c s) -> p c s", c=KH)
                nc.gpsimd.dma_start(
                    out=snd_d[io].ap(),
                    in_=stv[:, :, (p * CH + 1) * BC:(p * CH + 1 + CH) * BC])
                nc.gpsimd.collective_compute(
                    "AllGather", mybir.AluOpType.bypass,
                    replica_groups=GROUPS,
                    ins=[snd_d[io].ap().opt()],
                    outs=[rcv_d[io].ap().opt()])
                nc.gpsimd.dma_start(out=rxT[p % 2][:],
                                    in_=rcv_d[io].ap()[0])
            xpb_cur = xpb_next

        nc.vector.tensor_reduce(
            out=pooled[:].rearrange("p (c t) -> p c t", c=KH),
            in_=st[:].rearrange("p (c s b) -> p c s b", c=KH, b=BC)
                [:, :, 2 * CH + 1:2 * CH + 1 + T, :],
            axis=mybir.AxisListType.X, op=OP.add)
        nc.sync.dma_start(out=out_d[:], in_=pooled[:])

    nc.compile()
    return nc


def _prep(inputs):
    bf = ml_dtypes.bfloat16

    def packT(W, nk):
        WT = np.ascontiguousarray(np.asarray(W, np.float32).T)
        K = WT.shape[0]
        pad = np.zeros((nk * 128, G3), np.float32)
        pad[:K] = WT
        return np.concatenate([pad[k * 128:(k + 1) * 128] for k in range(nk)],
                              axis=1).astype(bf)

    def phase_bias(bih, bhh, active):
        b = np.asarray(bih, np.float32).copy()
        b[:2 * H] += np.asarray(bhh, np.float32)[:2 * H]
        bm = np.ascontiguousarray(b.reshape(M3, 128).T)      # [128, 12]
        out = np.zeros((128, NPH * M3), np.float32)
        for p in range(NPH):
            if active[p]:
                out[:, p * M3:(p + 1) * M3] = bm
        return out

    texts = np.asarray(inputs["texts"])
    z3g = np.zeros((128, EK * G3), bf)
    z4g = np.zeros((128, KH * G3), bf)
    zidx = np.zeros((128, NBLK), np.int32)
    in_maps = []
    for c in range(NCORE):
        l0 = c < DP
        q = c % DP
        act = [p < NCH for p in range(NPH)] if l0 else \
              [p >= 2 for p in range(NPH)]
        Wih, Whh = (inputs["Wih0"], inputs["Whh0"]) if l0 else \
                   (inputs["Wih1"], inputs["Whh1"])
        bih, bhh = (inputs["bih0"], inputs["bhh0"]) if l0 else \
                   (inputs["bih1"], inputs["bhh1"])
        bnr = np.zeros((128, KH * 128), bf)
        bnr[0, :] = np.asarray(bhh, np.float32)[2 * H:].astype(bf)
        pgm = np.zeros((128, NPH * BC), bf)
        for p in range(NPH):
            if act[p]:
                pgm[0, p * BC:(p + 1) * BC] = 1.0
        idxc = np.ascontiguousarray(
            texts[:, q * BC:(q + 1) * BC].astype(np.int32)
            .reshape(NBLK, 128).T) if l0 else zidx
        in_maps.append({
            "emb": np.ascontiguousarray(inputs["emb"], dtype=np.float32),
            "idx": idxc,
            "whh": packT(Whh, KH),
            "wia": packT(Wih, EK) if l0 else z3g,
            "wib": packT(Wih, KH) if not l0 else z4g,
            "b0": phase_bias(bih, bhh, act),
            "bn": bnr,
            "pg": pgm,
        })
    return in_maps


def _postproc(results):
    s = np.zeros((128, KH * T), np.float32)
    for c in range(DP, NCORE):
        s += results[c]["out"]
    return s.reshape(128, KH, T).transpose(2, 1, 0).reshape(T, H) / B


def kernel(**inputs):
    from concourse import bass_utils
    if "nc" not in _cache:
        _cache["nc"] = _build()
    nc = _cache["nc"]
    in_maps = _prep(inputs)
    res = bass_utils.run_bass_kernel_spmd(
        nc, in_maps, core_ids=list(range(NCORE)))
    pooled = _postproc([res.results[i] for i in range(NCORE)])
    fc_W = np.asarray(inputs["fc_W"], dtype=np.float32)
    fc_b = np.asarray(inputs["fc_b"], dtype=np.float32)
    return (pooled @ fc_W.T + fc_b).astype(np.float32)


if __name__ == "__main__":
    import time
    t0 = time.time()
    nc = _build()
    print("build+compile time:", round(time.time() - t0, 1), "s")
